# revision 39
# baseline (speedup 1.0000x reference)
"""Trainium2 Bass kernel for nn_JointNetwork (RNN-T joint: broadcast-add + 2-layer MLP).

Key insight: the module is fully linear (no activation between the Dense layers):
    out[b,t,u,:] = (enc[b,t]+pred[b,u]) @ W0 @ W1 + b0 @ W1 + b1
                 = E'[b,t,:] + P'[b,u,:]
with E' = enc@W0@W1 + b0@W1 + b1  (shape [B,T,V], small)
     P' = pred@W0@W1              (shape [B,U,V], small)
So the 206-GFLOP einsum collapses to tiny matmuls plus a broadcast-add whose
cost is purely the 512 MB HBM write of the output -> memory roofline.

Sharding: 8 cores, core c handles b = c//4, t-range [(c%4)*128, (c%4)*128+128).

v1 layout (vs the earlier u-on-partitions version): the 128 t-rows of each
core sit on the SBUF partition dim, so the output DMA target
out[t, u0:u0+C, :] is one CONTIGUOUS C*V*4 = 64 KB run per partition --
128 big descriptors per 8 MB dma_start instead of 1024 scattered 4 KB ones.
Phase B broadcasts P' rows across partitions via PE outer products -- two
exact bf16 products (P = P_hi + P_lo) accumulated in fp32 PSUM, at the full
1 cycle/row PE rate (plain fp32 would be 4 cyc/row; fp32r keeps only ~12-13
mantissa bits, HW-probed, and fails the accuracy gate) -- DVE adds E_s, sync
queue streams 8 x 8 MB stores.  Inputs arrive pre-transposed/pre-swizzled
from the host (pure layout prep), so there is no on-device transpose phase.

Raw Bass (no TileContext): this container's walrus build rejects instructions
with >1 sync-wait, which TileContext's scheduler emits. All synchronization is
explicit single-wait semaphores.

`_build_nc(reps=R)` unrolls R full kernel bodies (reload inputs, recompute,
rewrite the full output) inside one NEFF; `_timed_run` uses the marginal time
between an R-rep and a 1-rep NEFF across N pipelined dispatches to measure
per-execution HW time with the ~70 ms axon-tunnel RTT cancelled out.
"""

import os
import sys

if "/opt/trn_rl_repo" not in sys.path:
    sys.path.insert(0, "/opt/trn_rl_repo")

import numpy as np

B, T, U, D, H, V = 2, 512, 128, 512, 512, 1024
NCORES = 8
ROWS = 128          # t rows per core
C = 16              # u columns per output DMA group (8 MB per dma_start)
NGROUPS = U // C    # 8
KD = D // 128
KH = H // 128
NV = V // 512

_cache = {}


def _build_nc(reps=1):
    import concourse.bass as bass
    import concourse.mybir as mybir
    from contextlib import ExitStack

    fp32 = mybir.dt.float32
    bf16 = mybir.dt.bfloat16
    # Numerics (HW-probed): fp32r matmuls keep only ~12-13 mantissa bits
    # (4.8e-4 absmax even for exact 1.0*x broadcast products) -> fails the
    # 2e-2 gate at cancellation elements.  bf16 broadcasts are bit-exact.
    # So the prep chain (E1T/P1T/E'/P') runs in plain fp32 (4 cyc/row, but
    # tiny), and phase B broadcasts P' = P_hi + P_lo as TWO exact bf16
    # outer products accumulating in fp32 PSUM: error ~|P|*2^-17, and the
    # PE runs at the full 1 cyc/row bf16 rate.
    nc = bass.Bass()

    epT_d = nc.dram_tensor("epT", [128, KD * 256], fp32, kind="ExternalInput")
    w0_d = nc.dram_tensor("w0", [128, KD * H], fp32, kind="ExternalInput")
    w1_d = nc.dram_tensor("w1", [128, KH * V], fp32, kind="ExternalInput")
    b0t_d = nc.dram_tensor("b0t", [128, KH], fp32, kind="ExternalInput")
    b1_d = nc.dram_tensor("b1", [1, V], fp32, kind="ExternalInput")
    # bf16 constants loaded from DRAM: 4 delta-interleave matrices that remap
    # P' hi/lo rows onto adjacent partitions (inter[0]: j=2k<-Ph[k<64],
    # inter[1]: j=2k+1<-Pl[k<64], inter[2/3]: same for k>=64), and the [2,128]
    # all-ones stationary shared by EVERY phase-B broadcast matmul.
    inter_d = nc.dram_tensor("inter", [128, 4 * 128], bf16, kind="ExternalInput")
    twohot_d = nc.dram_tensor("twohot", [128, 64], bf16, kind="ExternalInput")
    ones_d = nc.dram_tensor("ones", [1, 128], fp32, kind="ExternalInput")
    out_d = nc.dram_tensor("out", [ROWS, U, V], fp32, kind="ExternalOutput")

    with ExitStack() as st:
        def sb(name, shape, dt=fp32):
            return st.enter_context(nc.sbuf_tensor(name, shape, dt))

        # epT_s[p,k,0:128] = enc[t, k*128+p]; [p,k,128+u] = pred[u, k*128+p]
        epT_s = sb("epT_s", [128, KD, 256])
        w0_s = sb("w0_s", [128, KD, H])                 # w0_s[p,k,h] = W0[k*128+p, h]
        w1_s = sb("w1_s", [128, KH, V])
        b0t_s = sb("b0t_s", [128, KH])                  # b0t_s[p,k]  = b0[k*128+p]
        b1_s = sb("b1_s", [1, V])
        ones_s = sb("ones_s", [1, 128])
        inter_s = sb("inter_s", [128, 4, 128], bf16)
        twohot_s = sb("twohot_s", [128, 64], bf16)
        e1t_s = sb("e1t_s", [128, KH, ROWS])            # e1t[p,k,t] = (enc@W0+b0)[t, k*128+p]
        p1t_s = sb("p1t_s", [128, KH, U])
        E_s = sb("E_s", [128, V])                       # E'[t, v]
        Ph_s = sb("Ph_s", [128, V], bf16)               # P' split: P ~= Ph + Pl
        Pl_s = sb("Pl_s", [128, V], bf16)
        # interleaved pairs: Pab[g][2i] = Ph[64g+i], Pab[g][2i+1] = Pl[64g+i]
        Pab_s = [sb(f"Pab{i}", [128, V], bf16) for i in range(2)]
        obuf = [sb(f"obuf{i}", [128, C, V]) for i in range(2)]
        psum = [
            st.enter_context(nc.psum_tensor(f"ps{i}", [128, V], fp32))
            for i in range(4)
        ]

        dma_in = st.enter_context(nc.semaphore("dma_in"))
        pe_prep = st.enter_context(nc.semaphore("pe_prep"))
        cp_sem = st.enter_context(nc.semaphore("cp"))
        pe_done = st.enter_context(nc.semaphore("pe_done"))
        dve_done = st.enter_context(nc.semaphore("dve_done"))
        dma_out = st.enter_context(nc.semaphore("dma_out"))

        blk = st.enter_context(nc.Block())

        # ---- input loads on the scalar HWDGE queue (doesn't block output
        # stores on the sync queue)
        @blk.scalar
        def _(sc):
            sc.dma_start(inter_s[:], inter_d[:]).then_inc(dma_in, 16)     # 16
            sc.dma_start(twohot_s[:], twohot_d[:]).then_inc(dma_in, 16)
            sc.dma_start(ones_s[:], ones_d[:]).then_inc(dma_in, 16)       # 48->base 48
            for r in range(reps):
                db = 48 + 80 * r
                if r > 0:
                    # rep r-1 prep (all SBUF input reads) finished
                    sc.wait_ge(pe_prep, 8 * r)
                sc.dma_start(epT_s[:], epT_d[:]).then_inc(dma_in, 16)     # db+16
                sc.dma_start(w0_s[:], w0_d[:]).then_inc(dma_in, 16)       # db+32
                sc.dma_start(w1_s[:], w1_d[:]).then_inc(dma_in, 16)       # db+48
                sc.dma_start(b0t_s[:], b0t_d[:]).then_inc(dma_in, 16)     # db+64
                sc.dma_start(b1_s[:], b1_d[:]).then_inc(dma_in, 16)       # db+80

        # ---- output stores on the sync HWDGE queue
        @blk.sync
        def _(s):
            for r in range(reps):
                for g in range(NGROUPS):
                    gg = r * NGROUPS + g
                    s.wait_ge(dve_done, 128 * r + C * (g + 1))
                    s.dma_start(
                        out_d[:, g * C:(g + 1) * C, :], obuf[gg % 2][:]
                    ).then_inc(dma_out, 16)
            s.wait_ge(dma_out, 16 * NGROUPS * reps)

        @blk.tensor
        def _(pe):
            for r in range(reps):
                db = 48 + 80 * r     # dma_in base (48 = inter+ones2+ones)
                cb = 13 * r      # cp_sem base
                pb = 8 * r       # pe_prep base
                if r > 0:
                    pe.wait_ge(dve_done, 128 * r)   # all rep r-1 psum reads done
                # --- fused [E1T | P1T][h, 0:256], psum[2+hb%2][:, 0:256]
                #     E1T[h,t] = sum_d W0[d,h] enc[t,d]; P1T[h,u] likewise
                pe.wait_ge(dma_in, db + 32)          # epT + w0 loaded
                for hb in range(KH):
                    if hb >= 2:
                        # DVE finished both copies of psum[2+hb%2][:, 0:256]
                        pe.wait_ge(cp_sem, cb + 2 * (hb - 2) + 2)
                    for k in range(KD):
                        ins = pe.matmul(
                            psum[2 + hb % 2][:, 0:256],
                            w0_s[:, k, hb * 128:(hb + 1) * 128],
                            epT_s[:, k, :],
                            start=(k == 0), stop=(k == KD - 1),
                        )
                    ins.then_inc(pe_prep, 1)         # pe_prep pb+1..4
                # --- E' = e1t^T @ W1 + ones^T @ b1 -> psum[0] (both banks)
                pe.wait_ge(cp_sem, cb + 7)           # e1t copies done
                pe.wait_ge(dma_in, db + 80)          # w1 + b1 loaded
                for vc in range(NV):
                    for hb in range(KH):
                        pe.matmul(
                            psum[0][:, vc * 512:(vc + 1) * 512],
                            e1t_s[:, hb, :],
                            w1_s[:, hb, vc * 512:(vc + 1) * 512],
                            start=(hb == 0), stop=False,
                        )
                    ins = pe.matmul(
                        psum[0][:, vc * 512:(vc + 1) * 512],
                        ones_s[:],
                        b1_s[0:1, vc * 512:(vc + 1) * 512],
                        start=False, stop=True,
                    )
                ins.then_inc(pe_prep, 1)             # pe_prep pb+5
                # --- P' -> psum[1]
                pe.wait_ge(cp_sem, cb + 8)           # p1t copies done
                for vc in range(NV):
                    for hb in range(KH):
                        ins = pe.matmul(
                            psum[1][:, vc * 512:(vc + 1) * 512],
                            p1t_s[:, hb, :],
                            w1_s[:, hb, vc * 512:(vc + 1) * 512],
                            start=(hb == 0), stop=(hb == KH - 1),
                        )
                ins.then_inc(pe_prep, 1)             # pe_prep pb+6
                # --- interleave P' hi/lo onto adjacent partitions:
                #     Pab[g][2i] = Ph[64g+i], Pab[g][2i+1] = Pl[64g+i]
                #     (delta stationaries; all products exact, psum->bf16
                #     copy exact since values are already bf16)
                pe.wait_ge(cp_sem, cb + 11)          # Ph + Pl in SBUF
                for g in range(2):
                    for vc in range(NV):
                        pe.matmul(
                            psum[2 + g][:, vc * 512:(vc + 1) * 512],
                            inter_s[:, 2 * g, :],
                            Ph_s[:, vc * 512:(vc + 1) * 512],
                            start=True, stop=False,
                        )
                        ins = pe.matmul(
                            psum[2 + g][:, vc * 512:(vc + 1) * 512],
                            inter_s[:, 2 * g + 1, :],
                            Pl_s[:, vc * 512:(vc + 1) * 512],
                            start=False, stop=True,
                        )
                    ins.then_inc(pe_prep, 1)         # pe_prep pb+7, pb+8
                # --- phase B: broadcast each P' row across 128 partitions:
                #     ONE matmul per chunk -- two-hot stationary column
                #     e_{2u'} + e_{2u'+1} sums the interleaved hi/lo pair:
                #     out[t,v] = Pab[2u'] + Pab[2u'+1] = Ph[u]+Pl[u] = P'[u]
                pe.wait_ge(cp_sem, cb + 13)          # E_s + Pab in SBUF
                for u in range(U):
                    if u >= 4:
                        pe.wait_ge(dve_done, 128 * r + u - 3)
                    sel = twohot_s[:, (u % 64):(u % 64) + 1].broadcast_to([128, 128])
                    src = Pab_s[u // 64]
                    for vc in range(NV):
                        ins = pe.matmul(
                            psum[u % 4][:, vc * 512:(vc + 1) * 512],
                            sel,
                            src[:, vc * 512:(vc + 1) * 512],
                            start=True, stop=True,
                        )
                    ins.then_inc(pe_done, 1)         # pe_done 128r+u+1

        @blk.vector
        def _(v):
            for r in range(reps):
                db = 48 + 80 * r
                pb = 8 * r
                # per hb: e1t with b0 bias from psum[.., 0:128], p1t copy from
                # psum[.., 128:256]
                v.wait_ge(dma_in, db + 64)           # b0t loaded
                for hb in range(KH):
                    v.wait_ge(pe_prep, pb + 1 + hb)
                    v.tensor_scalar_add(
                        e1t_s[:, hb, :], psum[2 + hb % 2][:, 0:128],
                        b0t_s[:, hb:hb + 1],
                    ).then_inc(cp_sem, 1)            # cp cb+2hb+1
                    v.tensor_copy(
                        p1t_s[:, hb, :], psum[2 + hb % 2][:, 128:256]
                    ).then_inc(cp_sem, 1)            # cp cb+2hb+2
                v.wait_ge(pe_prep, pb + 5)
                v.tensor_copy(E_s[:], psum[0][:]).then_inc(cp_sem, 1)   # cb+9
                v.wait_ge(pe_prep, pb + 6)
                v.tensor_copy(Ph_s[:], psum[1][:]).then_inc(cp_sem, 1)  # cb+10
                v.tensor_sub(Pl_s[:], psum[1][:], Ph_s[:]).then_inc(cp_sem, 1)  # cb+11
                for g in range(2):                   # psum->bf16 exact copies
                    v.wait_ge(pe_prep, pb + 7 + g)
                    v.tensor_copy(Pab_s[g][:], psum[2 + g][:]
                                  ).then_inc(cp_sem, 1)  # cb+12, cb+13
                # --- phase B adds
                for u in range(U):
                    gg = r * NGROUPS + u // C
                    if u % C == 0 and gg >= 2:
                        v.wait_ge(dma_out, 16 * (gg - 1))
                    v.wait_ge(pe_done, 128 * r + u + 1)
                    v.tensor_add(
                        obuf[gg % 2][:, u % C, :], psum[u % 4][:], E_s[:]
                    ).then_inc(dve_done, 1)

    return nc


def _in_maps(pred_inp, enc_inp, W0, b0, W1, b1):
    import ml_dtypes

    def swiz(m, kb):
        # [kb*128, X] -> [128, kb, X] with row p holding blocks k
        return np.ascontiguousarray(
            m.reshape(kb, 128, m.shape[1]).transpose(1, 0, 2), dtype=np.float32
        )

    w0s = swiz(np.asarray(W0, np.float32), KD).reshape(128, -1)
    w1s = swiz(np.asarray(W1, np.float32), KH).reshape(128, -1)
    b0t = np.ascontiguousarray(
        np.asarray(b0, np.float32).reshape(KH, 128).T, dtype=np.float32
    )
    b1r = np.asarray(b1, np.float32).reshape(1, V)
    predT = {}
    for b in range(B):
        predT[b] = swiz(np.ascontiguousarray(np.asarray(pred_inp[b], np.float32).T), KD)
    maps = []
    for c in range(NCORES):
        b = c // 4
        t0 = (c % 4) * ROWS
        encT = swiz(
            np.ascontiguousarray(np.asarray(enc_inp[b, t0:t0 + ROWS, :], np.float32).T),
            KD,
        )
        epT = np.concatenate([encT, predT[b]], axis=2).reshape(128, -1)
        maps.append({
            "epT": np.ascontiguousarray(epT),
            "w0": w0s,
            "w1": w1s,
            "b0t": b0t,
            "b1": b1r,
            "inter": _inter_const(),
            "twohot": _twohot_const(),
            "ones": np.ones((1, 128), dtype=np.float32),
        })
    return maps


def _inter_const():
    import ml_dtypes

    inter = np.zeros((128, 4, 128), dtype=ml_dtypes.bfloat16)
    for k in range(64):
        inter[k, 0, 2 * k] = 1
        inter[k, 1, 2 * k + 1] = 1
    for k in range(64, 128):
        inter[k, 2, 2 * (k - 64)] = 1
        inter[k, 3, 2 * (k - 64) + 1] = 1
    return inter.reshape(128, -1)


def _twohot_const():
    import ml_dtypes

    m = np.zeros((128, 64), dtype=ml_dtypes.bfloat16)
    for up in range(64):
        m[2 * up, up] = 1
        m[2 * up + 1, up] = 1
    return m


def _run(pred_inp, enc_inp, W0, b0, W1, b1, trace=False):
    from concourse.bass_utils import run_bass_kernel_spmd

    if "nc" not in _cache:
        _cache["nc"] = _build_nc(reps=1)
    nc = _cache["nc"]
    res = run_bass_kernel_spmd(
        nc, _in_maps(pred_inp, enc_inp, W0, b0, W1, b1),
        list(range(NCORES)), trace=trace,
    )
    out = np.empty((B, T, U, V), dtype=np.float32)
    for c in range(NCORES):
        b = c // 4
        t0 = (c % 4) * ROWS
        out[b, t0:t0 + ROWS] = res.results[c]["out"]
    return out, res


def _gather(out_concat):
    res0 = np.asarray(out_concat).reshape(NCORES, ROWS, U, V)
    full = np.empty((B, T, U, V), dtype=np.float32)
    for c in range(NCORES):
        b = c // 4
        t0 = (c % 4) * ROWS
        full[b, t0:t0 + ROWS] = res0[c]
    return full


def kernel(pred_inp, enc_inp, W0, b0, W1, b1):
    """Full-input, full-output entry point (8-core SPMD inside).

    Dispatches twice and returns the second result: the very first NEFF
    execution after load intermittently corrupts whole core-shards (HW
    cold-start quirk, observed & characterized on-device); executions >= 1
    are deterministic and bit-identical.
    """
    import jax
    from concourse import bass2jax

    bass2jax.install_neuronx_cc_hook()
    maps = _in_maps(pred_inp, enc_inp, W0, b0, W1, b1)
    if "nc1" not in _cache:
        _cache["nc1"] = _build_nc(reps=1)
    if "fn1" not in _cache:
        _cache["fn1"] = _make_sharded(_cache["nc1"])
    fn, in_names, zero_outs, mesh, P = _cache["fn1"]
    sh = jax.sharding.NamedSharding(mesh, P)
    concat_in = [
        jax.device_put(
            np.concatenate([maps[c][nm] for c in range(NCORES)], axis=0), sh
        )
        for nm in in_names
    ]
    cur = [
        jax.device_put(np.zeros((NCORES * z.shape[0], *z.shape[1:]), z.dtype), sh)
        for z in zero_outs
    ]
    jax.block_until_ready(concat_in)
    jax.block_until_ready(cur)
    cur = list(fn(*concat_in, *cur))   # warmup (cold-start exec, discarded)
    cur = list(fn(*concat_in, *cur))   # second warmup, belt and braces
    cur = list(fn(*concat_in, *cur))
    jax.block_until_ready(cur)
    return _gather(cur[0])


def _make_sharded(nc):
    """jit(shard_map(bass_exec)) for `nc` on 8 cores; returns (fn, in_names,
    zero_outs, mesh, P)."""
    import jax
    from concourse import bass2jax, mybir

    in_names, out_names, out_avals, zero_outs = [], [], [], []
    for alloc in nc.m.functions[0].allocations:
        if not isinstance(alloc, mybir.MemoryLocationSet):
            continue
        name = alloc.memorylocations[0].name
        pname = nc.partition_id_tensor.name if nc.partition_id_tensor else None
        if alloc.kind == "ExternalInput":
            if name != pname:
                in_names.append(name)
        elif alloc.kind == "ExternalOutput":
            out_names.append(name)
            shape = tuple(alloc.tensor_shape)
            dt = mybir.dt.np(alloc.dtype)
            out_avals.append(jax.core.ShapedArray(shape, dt))
            zero_outs.append(np.zeros(shape, dt))
    n_params = len(in_names)
    all_names = in_names + out_names
    if nc.partition_id_tensor is not None:
        all_names = all_names + [nc.partition_id_tensor.name]

    def _body(*args):
        operands = list(args)
        if nc.partition_id_tensor is not None:
            operands.append(bass2jax.partition_id_tensor())
        outs = bass2jax._bass_exec_p.bind(
            *operands,
            out_avals=tuple(out_avals),
            in_names=tuple(all_names),
            out_names=tuple(out_names),
            lowering_input_output_aliases=(),
            sim_require_finite=True,
            sim_require_nnan=True,
            nc=nc,
        )
        return tuple(outs)

    devices = jax.devices()[:NCORES]
    mesh = bass2jax.Mesh(np.asarray(devices), ("core",))
    P = bass2jax.PartitionSpec("core")
    # PJRT allocates bass_exec custom-call results uninitialized; donating
    # the output operands lets XLA alias them to the results so the NEFF's
    # writes land in the returned buffers (same mechanism run_bass_via_pjrt
    # uses).  Callers chain each dispatch's outputs into the next call's
    # output operands, so no fresh zero buffers are ever uploaded.
    fn = jax.jit(
        bass2jax.shard_map(
            _body, mesh=mesh, in_specs=(P,) * (n_params + len(out_names)),
            out_specs=(P,) * len(out_names), check_rep=False,
        ),
        donate_argnums=tuple(range(n_params, n_params + len(out_names))),
        keep_unused=True,
    )
    return fn, in_names, zero_outs, mesh, P


def _timed_run(pred_inp, enc_inp, W0, b0, W1, b1, reps_inner=33, n_disp=32,
               outer=5):
    """Measure per-execution HW time of the kernel through the axon tunnel.

    The tunnel RTT (~50-100 ms) dwarfs the on-device execution, so a single
    dispatch wall-clock measures the network, not the kernel.  Instead:
    compile two NEFFs -- one with `reps_inner` unrolled kernel bodies, one
    with a single body -- pipeline `n_disp` async dispatches of each, and
    take the marginal time per extra body:

        exec_ns = (T[R reps] - T[1 rep]) / (n_disp * (R - 1))

    Both T's carry identical RTT + per-dispatch overhead, which cancels.
    Every body does the full job: loads inputs from HBM, computes E'/P',
    broadcasts, and writes the entire 64 MB output shard.

    Returns (full_output, exec_ns).
    """
    import time
    import jax
    from concourse import bass2jax

    bass2jax.install_neuronx_cc_hook()

    maps = _in_maps(pred_inp, enc_inp, W0, b0, W1, b1)
    timings = {}
    outs_np = None
    sh = None
    concat_in = cur = None
    for reps in (1, reps_inner):
        key = f"nc{reps}"
        if key not in _cache:
            _cache[key] = _build_nc(reps=reps)
        nc = _cache[key]
        fkey = f"fn{reps}"
        if fkey not in _cache:
            _cache[fkey] = _make_sharded(nc)
        fn, in_names, zero_outs, mesh, P = _cache[fkey]
        if sh is None:
            sh = jax.sharding.NamedSharding(mesh, P)
            concat_in = [
                jax.device_put(
                    np.concatenate([maps[c][nm] for c in range(NCORES)], axis=0),
                    sh,
                )
                for nm in in_names
            ]
            # initial donated output operands; every later dispatch donates
            # the previous dispatch's outputs (the kernel writes every
            # element, so initial contents are irrelevant)
            cur = [
                jax.device_put(
                    np.zeros((NCORES * z.shape[0], *z.shape[1:]), z.dtype), sh
                )
                for z in zero_outs
            ]
            jax.block_until_ready(concat_in)
            jax.block_until_ready(cur)
        # warmup: compile + two execs (first-after-load is unreliable, see
        # kernel() docstring) + correctness snapshot from 1-rep
        cur = list(fn(*concat_in, *cur))
        cur = list(fn(*concat_in, *cur))
        jax.block_until_ready(cur)
        if reps == 1:
            outs_np = np.asarray(cur[0])
        best = None
        for _ in range(outer):
            t0 = time.perf_counter()
            for _i in range(n_disp):
                cur = list(fn(*concat_in, *cur))
            jax.block_until_ready(cur)
            dt = time.perf_counter() - t0
            best = dt if best is None else min(best, dt)
        timings[reps] = best
        if os.environ.get("TIME_DEBUG"):
            print(f"  reps={reps}: best total {best*1e3:.2f} ms "
                  f"({best/n_disp*1e3:.3f} ms/dispatch)")

    exec_ns = (timings[reps_inner] - timings[1]) / (n_disp * (reps_inner - 1)) * 1e9
    return _gather(outs_np), int(exec_ns)


# revision 40
# speedup vs baseline: 1.1515x; 1.1515x over previous
"""Trainium2 Bass kernel for nn_JointNetwork (RNN-T joint: broadcast-add + 2-layer MLP).

Key insight: the module is fully linear (no activation between the Dense layers):
    out[b,t,u,:] = (enc[b,t]+pred[b,u]) @ W0 @ W1 + b0 @ W1 + b1
                 = E'[b,t,:] + P'[b,u,:]
with E' = enc@W0@W1 + b0@W1 + b1  (shape [B,T,V], small)
     P' = pred@W0@W1              (shape [B,U,V], small)
So the 206-GFLOP einsum collapses to tiny matmuls plus a broadcast-add whose
cost is purely the 512 MB HBM write of the output -> memory roofline.

Sharding: 8 cores, core c handles b = c//4, t-range [(c%4)*128, (c%4)*128+128).

v1 layout (vs the earlier u-on-partitions version): the 128 t-rows of each
core sit on the SBUF partition dim, so the output DMA target
out[t, u0:u0+C, :] is one CONTIGUOUS C*V*4 = 64 KB run per partition --
128 big descriptors per 8 MB dma_start instead of 1024 scattered 4 KB ones.
Phase B broadcasts P' rows across partitions via PE outer products.  P' is
split into exact bf16 hi/lo planes (fp32r keeps only ~12-13 mantissa bits,
HW-probed, and fails the accuracy gate; bf16 products are exact), the planes
are interleaved onto adjacent partitions (Pab[2i]=Ph[i], Pab[2i+1]=Pl[i])
once per rep by constant delta-matmuls, and each broadcast is then ONE
1-cyc/row bf16 matmul per 512-chunk whose two-hot stationary column
e_{2u'}+e_{2u'+1} sums hi+lo in fp32 PSUM.  DVE adds E_s, sync queue streams
8 x 8 MB stores.  Inputs arrive pre-transposed/pre-swizzled from the host
(pure layout prep), so there is no on-device transpose phase.  Measured at
~195 us/body marginal = ~350 GB/s of the ~358 GB/s per-core HBM limit for
the 67.5 MB/rep of traffic -- the memory roofline for this problem.

Raw Bass (no TileContext): this container's walrus build rejects instructions
with >1 sync-wait, which TileContext's scheduler emits. All synchronization is
explicit single-wait semaphores.

`_build_nc(reps=R)` unrolls R full kernel bodies (reload inputs, recompute,
rewrite the full output) inside one NEFF; `_timed_run` uses the marginal time
between an R-rep and a 1-rep NEFF across N pipelined dispatches to measure
per-execution HW time with the ~70 ms axon-tunnel RTT cancelled out.
"""

import os
import sys

if "/opt/trn_rl_repo" not in sys.path:
    sys.path.insert(0, "/opt/trn_rl_repo")

import numpy as np

B, T, U, D, H, V = 2, 512, 128, 512, 512, 1024
NCORES = 8
ROWS = 128          # t rows per core
C = 16              # u columns per output DMA group (8 MB per dma_start)
NGROUPS = U // C    # 8
KD = D // 128
KH = H // 128
NV = V // 512

_cache = {}


def _build_nc(reps=1):
    import concourse.bass as bass
    import concourse.mybir as mybir
    from contextlib import ExitStack

    fp32 = mybir.dt.float32
    bf16 = mybir.dt.bfloat16
    # Numerics (HW-probed): fp32r matmuls keep only ~12-13 mantissa bits
    # (4.8e-4 absmax even for exact 1.0*x broadcast products) -> fails the
    # 2e-2 gate at cancellation elements.  bf16 broadcasts are bit-exact.
    # So the prep chain (E1T/P1T/E'/P') runs in plain fp32 (4 cyc/row, but
    # tiny), and phase B broadcasts P' = P_hi + P_lo as TWO exact bf16
    # outer products accumulating in fp32 PSUM: error ~|P|*2^-17, and the
    # PE runs at the full 1 cyc/row bf16 rate.
    nc = bass.Bass()

    epT_d = nc.dram_tensor("epT", [128, KD * 256], fp32, kind="ExternalInput")
    w0_d = nc.dram_tensor("w0", [128, KD * H], fp32, kind="ExternalInput")
    w1_d = nc.dram_tensor("w1", [128, KH * V], fp32, kind="ExternalInput")
    b0t_d = nc.dram_tensor("b0t", [128, KH], fp32, kind="ExternalInput")
    b1_d = nc.dram_tensor("b1", [1, V], fp32, kind="ExternalInput")
    # bf16 constants loaded from DRAM: 4 delta-interleave matrices that remap
    # P' hi/lo rows onto adjacent partitions (inter[0]: j=2k<-Ph[k<64],
    # inter[1]: j=2k+1<-Pl[k<64], inter[2/3]: same for k>=64), and the [2,128]
    # all-ones stationary shared by EVERY phase-B broadcast matmul.
    inter_d = nc.dram_tensor("inter", [128, 4 * 128], bf16, kind="ExternalInput")
    twohot_d = nc.dram_tensor("twohot", [128, 64], bf16, kind="ExternalInput")
    ones_d = nc.dram_tensor("ones", [1, 128], fp32, kind="ExternalInput")
    out_d = nc.dram_tensor("out", [ROWS, U, V], fp32, kind="ExternalOutput")

    with ExitStack() as st:
        def sb(name, shape, dt=fp32):
            return st.enter_context(nc.sbuf_tensor(name, shape, dt))

        # epT_s[p,k,0:128] = enc[t, k*128+p]; [p,k,128+u] = pred[u, k*128+p]
        epT_s = sb("epT_s", [128, KD, 256])
        w0_s = sb("w0_s", [128, KD, H])                 # w0_s[p,k,h] = W0[k*128+p, h]
        w1_s = sb("w1_s", [128, KH, V])
        b0t_s = sb("b0t_s", [128, KH])                  # b0t_s[p,k]  = b0[k*128+p]
        b1_s = sb("b1_s", [1, V])
        ones_s = sb("ones_s", [1, 128])
        inter_s = sb("inter_s", [128, 4, 128], bf16)
        twohot_s = sb("twohot_s", [128, 64], bf16)
        e1t_s = sb("e1t_s", [128, KH, ROWS])            # e1t[p,k,t] = (enc@W0+b0)[t, k*128+p]
        p1t_s = sb("p1t_s", [128, KH, U])
        E_s = sb("E_s", [128, V])                       # E'[t, v]
        Ph_s = sb("Ph_s", [128, V], bf16)               # P' split: P ~= Ph + Pl
        Pl_s = sb("Pl_s", [128, V], bf16)
        # interleaved pairs: Pab[g][2i] = Ph[64g+i], Pab[g][2i+1] = Pl[64g+i]
        Pab_s = [sb(f"Pab{i}", [128, V], bf16) for i in range(2)]
        obuf = [sb(f"obuf{i}", [128, C, V]) for i in range(2)]
        psum = [
            st.enter_context(nc.psum_tensor(f"ps{i}", [128, V], fp32))
            for i in range(4)
        ]

        dma_in = st.enter_context(nc.semaphore("dma_in"))
        pe_prep = st.enter_context(nc.semaphore("pe_prep"))
        cp_sem = st.enter_context(nc.semaphore("cp"))
        pe_done = st.enter_context(nc.semaphore("pe_done"))
        dve_done = st.enter_context(nc.semaphore("dve_done"))
        dma_out = st.enter_context(nc.semaphore("dma_out"))

        blk = st.enter_context(nc.Block())

        # ---- input loads on the scalar HWDGE queue (doesn't block output
        # stores on the sync queue)
        @blk.scalar
        def _(sc):
            sc.dma_start(inter_s[:], inter_d[:]).then_inc(dma_in, 16)     # 16
            sc.dma_start(twohot_s[:], twohot_d[:]).then_inc(dma_in, 16)
            sc.dma_start(ones_s[:], ones_d[:]).then_inc(dma_in, 16)       # 48->base 48
            for r in range(reps):
                db = 48 + 80 * r
                if r > 0:
                    # rep r-1 prep (all SBUF input reads) finished
                    sc.wait_ge(pe_prep, 8 * r)
                sc.dma_start(epT_s[:], epT_d[:]).then_inc(dma_in, 16)     # db+16
                sc.dma_start(w0_s[:], w0_d[:]).then_inc(dma_in, 16)       # db+32
                sc.dma_start(w1_s[:], w1_d[:]).then_inc(dma_in, 16)       # db+48
                sc.dma_start(b0t_s[:], b0t_d[:]).then_inc(dma_in, 16)     # db+64
                sc.dma_start(b1_s[:], b1_d[:]).then_inc(dma_in, 16)       # db+80

        # ---- output stores on the sync HWDGE queue
        @blk.sync
        def _(s):
            for r in range(reps):
                for g in range(NGROUPS):
                    gg = r * NGROUPS + g
                    s.wait_ge(dve_done, 128 * r + C * (g + 1))
                    s.dma_start(
                        out_d[:, g * C:(g + 1) * C, :], obuf[gg % 2][:]
                    ).then_inc(dma_out, 16)
            s.wait_ge(dma_out, 16 * NGROUPS * reps)

        @blk.tensor
        def _(pe):
            for r in range(reps):
                db = 48 + 80 * r     # dma_in base (48 = inter+ones2+ones)
                cb = 13 * r      # cp_sem base
                pb = 8 * r       # pe_prep base
                if r > 0:
                    pe.wait_ge(dve_done, 128 * r)   # all rep r-1 psum reads done
                # --- fused [E1T | P1T][h, 0:256], psum[2+hb%2][:, 0:256]
                #     E1T[h,t] = sum_d W0[d,h] enc[t,d]; P1T[h,u] likewise
                pe.wait_ge(dma_in, db + 32)          # epT + w0 loaded
                for hb in range(KH):
                    if hb >= 2:
                        # DVE finished both copies of psum[2+hb%2][:, 0:256]
                        pe.wait_ge(cp_sem, cb + 2 * (hb - 2) + 2)
                    for k in range(KD):
                        ins = pe.matmul(
                            psum[2 + hb % 2][:, 0:256],
                            w0_s[:, k, hb * 128:(hb + 1) * 128],
                            epT_s[:, k, :],
                            start=(k == 0), stop=(k == KD - 1),
                        )
                    ins.then_inc(pe_prep, 1)         # pe_prep pb+1..4
                # --- E' = e1t^T @ W1 + ones^T @ b1 -> psum[0] (both banks)
                pe.wait_ge(cp_sem, cb + 7)           # e1t copies done
                pe.wait_ge(dma_in, db + 80)          # w1 + b1 loaded
                for vc in range(NV):
                    for hb in range(KH):
                        pe.matmul(
                            psum[0][:, vc * 512:(vc + 1) * 512],
                            e1t_s[:, hb, :],
                            w1_s[:, hb, vc * 512:(vc + 1) * 512],
                            start=(hb == 0), stop=False,
                        )
                    ins = pe.matmul(
                        psum[0][:, vc * 512:(vc + 1) * 512],
                        ones_s[:],
                        b1_s[0:1, vc * 512:(vc + 1) * 512],
                        start=False, stop=True,
                    )
                ins.then_inc(pe_prep, 1)             # pe_prep pb+5
                # --- P' -> psum[1]
                pe.wait_ge(cp_sem, cb + 8)           # p1t copies done
                for vc in range(NV):
                    for hb in range(KH):
                        ins = pe.matmul(
                            psum[1][:, vc * 512:(vc + 1) * 512],
                            p1t_s[:, hb, :],
                            w1_s[:, hb, vc * 512:(vc + 1) * 512],
                            start=(hb == 0), stop=(hb == KH - 1),
                        )
                ins.then_inc(pe_prep, 1)             # pe_prep pb+6
                # --- interleave P' hi/lo onto adjacent partitions:
                #     Pab[g][2i] = Ph[64g+i], Pab[g][2i+1] = Pl[64g+i]
                #     (delta stationaries; all products exact, psum->bf16
                #     copy exact since values are already bf16)
                pe.wait_ge(cp_sem, cb + 11)          # Ph + Pl in SBUF
                for g in range(2):
                    for vc in range(NV):
                        pe.matmul(
                            psum[2 + g][:, vc * 512:(vc + 1) * 512],
                            inter_s[:, 2 * g, :],
                            Ph_s[:, vc * 512:(vc + 1) * 512],
                            start=True, stop=False,
                        )
                        ins = pe.matmul(
                            psum[2 + g][:, vc * 512:(vc + 1) * 512],
                            inter_s[:, 2 * g + 1, :],
                            Pl_s[:, vc * 512:(vc + 1) * 512],
                            start=False, stop=True,
                        )
                    ins.then_inc(pe_prep, 1)         # pe_prep pb+7, pb+8
                # --- phase B: broadcast each P' row across 128 partitions:
                #     ONE matmul per chunk -- two-hot stationary column
                #     e_{2u'} + e_{2u'+1} sums the interleaved hi/lo pair:
                #     out[t,v] = Pab[2u'] + Pab[2u'+1] = Ph[u]+Pl[u] = P'[u]
                pe.wait_ge(cp_sem, cb + 13)          # E_s + Pab in SBUF
                for u in range(U):
                    if u >= 4:
                        pe.wait_ge(dve_done, 128 * r + u - 3)
                    sel = twohot_s[:, (u % 64):(u % 64) + 1].broadcast_to([128, 128])
                    src = Pab_s[u // 64]
                    for vc in range(NV):
                        ins = pe.matmul(
                            psum[u % 4][:, vc * 512:(vc + 1) * 512],
                            sel,
                            src[:, vc * 512:(vc + 1) * 512],
                            start=True, stop=True,
                        )
                    ins.then_inc(pe_done, 1)         # pe_done 128r+u+1

        @blk.vector
        def _(v):
            for r in range(reps):
                db = 48 + 80 * r
                pb = 8 * r
                # per hb: e1t with b0 bias from psum[.., 0:128], p1t copy from
                # psum[.., 128:256]
                v.wait_ge(dma_in, db + 64)           # b0t loaded
                for hb in range(KH):
                    v.wait_ge(pe_prep, pb + 1 + hb)
                    v.tensor_scalar_add(
                        e1t_s[:, hb, :], psum[2 + hb % 2][:, 0:128],
                        b0t_s[:, hb:hb + 1],
                    ).then_inc(cp_sem, 1)            # cp cb+2hb+1
                    v.tensor_copy(
                        p1t_s[:, hb, :], psum[2 + hb % 2][:, 128:256]
                    ).then_inc(cp_sem, 1)            # cp cb+2hb+2
                v.wait_ge(pe_prep, pb + 5)
                v.tensor_copy(E_s[:], psum[0][:]).then_inc(cp_sem, 1)   # cb+9
                v.wait_ge(pe_prep, pb + 6)
                v.tensor_copy(Ph_s[:], psum[1][:]).then_inc(cp_sem, 1)  # cb+10
                v.tensor_sub(Pl_s[:], psum[1][:], Ph_s[:]).then_inc(cp_sem, 1)  # cb+11
                for g in range(2):                   # psum->bf16 exact copies
                    v.wait_ge(pe_prep, pb + 7 + g)
                    v.tensor_copy(Pab_s[g][:], psum[2 + g][:]
                                  ).then_inc(cp_sem, 1)  # cb+12, cb+13
                # --- phase B adds
                for u in range(U):
                    gg = r * NGROUPS + u // C
                    if u % C == 0 and gg >= 2:
                        v.wait_ge(dma_out, 16 * (gg - 1))
                    v.wait_ge(pe_done, 128 * r + u + 1)
                    v.tensor_add(
                        obuf[gg % 2][:, u % C, :], psum[u % 4][:], E_s[:]
                    ).then_inc(dve_done, 1)

    return nc


def _in_maps(pred_inp, enc_inp, W0, b0, W1, b1):
    import ml_dtypes

    def swiz(m, kb):
        # [kb*128, X] -> [128, kb, X] with row p holding blocks k
        return np.ascontiguousarray(
            m.reshape(kb, 128, m.shape[1]).transpose(1, 0, 2), dtype=np.float32
        )

    w0s = swiz(np.asarray(W0, np.float32), KD).reshape(128, -1)
    w1s = swiz(np.asarray(W1, np.float32), KH).reshape(128, -1)
    b0t = np.ascontiguousarray(
        np.asarray(b0, np.float32).reshape(KH, 128).T, dtype=np.float32
    )
    b1r = np.asarray(b1, np.float32).reshape(1, V)
    predT = {}
    for b in range(B):
        predT[b] = swiz(np.ascontiguousarray(np.asarray(pred_inp[b], np.float32).T), KD)
    maps = []
    for c in range(NCORES):
        b = c // 4
        t0 = (c % 4) * ROWS
        encT = swiz(
            np.ascontiguousarray(np.asarray(enc_inp[b, t0:t0 + ROWS, :], np.float32).T),
            KD,
        )
        epT = np.concatenate([encT, predT[b]], axis=2).reshape(128, -1)
        maps.append({
            "epT": np.ascontiguousarray(epT),
            "w0": w0s,
            "w1": w1s,
            "b0t": b0t,
            "b1": b1r,
            "inter": _inter_const(),
            "twohot": _twohot_const(),
            "ones": np.ones((1, 128), dtype=np.float32),
        })
    return maps


def _inter_const():
    import ml_dtypes

    inter = np.zeros((128, 4, 128), dtype=ml_dtypes.bfloat16)
    for k in range(64):
        inter[k, 0, 2 * k] = 1
        inter[k, 1, 2 * k + 1] = 1
    for k in range(64, 128):
        inter[k, 2, 2 * (k - 64)] = 1
        inter[k, 3, 2 * (k - 64) + 1] = 1
    return inter.reshape(128, -1)


def _twohot_const():
    import ml_dtypes

    m = np.zeros((128, 64), dtype=ml_dtypes.bfloat16)
    for up in range(64):
        m[2 * up, up] = 1
        m[2 * up + 1, up] = 1
    return m


def _run(pred_inp, enc_inp, W0, b0, W1, b1, trace=False):
    from concourse.bass_utils import run_bass_kernel_spmd

    if "nc" not in _cache:
        _cache["nc"] = _build_nc(reps=1)
    nc = _cache["nc"]
    res = run_bass_kernel_spmd(
        nc, _in_maps(pred_inp, enc_inp, W0, b0, W1, b1),
        list(range(NCORES)), trace=trace,
    )
    out = np.empty((B, T, U, V), dtype=np.float32)
    for c in range(NCORES):
        b = c // 4
        t0 = (c % 4) * ROWS
        out[b, t0:t0 + ROWS] = res.results[c]["out"]
    return out, res


def _gather(out_concat):
    res0 = np.asarray(out_concat).reshape(NCORES, ROWS, U, V)
    full = np.empty((B, T, U, V), dtype=np.float32)
    for c in range(NCORES):
        b = c // 4
        t0 = (c % 4) * ROWS
        full[b, t0:t0 + ROWS] = res0[c]
    return full


def kernel(pred_inp, enc_inp, W0, b0, W1, b1):
    """Full-input, full-output entry point (8-core SPMD inside).

    Dispatches twice and returns the second result: the very first NEFF
    execution after load intermittently corrupts whole core-shards (HW
    cold-start quirk, observed & characterized on-device); executions >= 1
    are deterministic and bit-identical.
    """
    import jax
    from concourse import bass2jax

    bass2jax.install_neuronx_cc_hook()
    maps = _in_maps(pred_inp, enc_inp, W0, b0, W1, b1)
    if "nc1" not in _cache:
        _cache["nc1"] = _build_nc(reps=1)
    if "fn1" not in _cache:
        _cache["fn1"] = _make_sharded(_cache["nc1"])
    fn, in_names, zero_outs, mesh, P = _cache["fn1"]
    sh = jax.sharding.NamedSharding(mesh, P)
    concat_in = [
        jax.device_put(
            np.concatenate([maps[c][nm] for c in range(NCORES)], axis=0), sh
        )
        for nm in in_names
    ]
    cur = [
        jax.device_put(np.zeros((NCORES * z.shape[0], *z.shape[1:]), z.dtype), sh)
        for z in zero_outs
    ]
    jax.block_until_ready(concat_in)
    jax.block_until_ready(cur)
    cur = list(fn(*concat_in, *cur))   # warmup (cold-start exec, discarded)
    cur = list(fn(*concat_in, *cur))   # second warmup, belt and braces
    cur = list(fn(*concat_in, *cur))
    jax.block_until_ready(cur)
    return _gather(cur[0])


def _make_sharded(nc):
    """jit(shard_map(bass_exec)) for `nc` on 8 cores; returns (fn, in_names,
    zero_outs, mesh, P)."""
    import jax
    from concourse import bass2jax, mybir

    in_names, out_names, out_avals, zero_outs = [], [], [], []
    for alloc in nc.m.functions[0].allocations:
        if not isinstance(alloc, mybir.MemoryLocationSet):
            continue
        name = alloc.memorylocations[0].name
        pname = nc.partition_id_tensor.name if nc.partition_id_tensor else None
        if alloc.kind == "ExternalInput":
            if name != pname:
                in_names.append(name)
        elif alloc.kind == "ExternalOutput":
            out_names.append(name)
            shape = tuple(alloc.tensor_shape)
            dt = mybir.dt.np(alloc.dtype)
            out_avals.append(jax.core.ShapedArray(shape, dt))
            zero_outs.append(np.zeros(shape, dt))
    n_params = len(in_names)
    all_names = in_names + out_names
    if nc.partition_id_tensor is not None:
        all_names = all_names + [nc.partition_id_tensor.name]

    def _body(*args):
        operands = list(args)
        if nc.partition_id_tensor is not None:
            operands.append(bass2jax.partition_id_tensor())
        outs = bass2jax._bass_exec_p.bind(
            *operands,
            out_avals=tuple(out_avals),
            in_names=tuple(all_names),
            out_names=tuple(out_names),
            lowering_input_output_aliases=(),
            sim_require_finite=True,
            sim_require_nnan=True,
            nc=nc,
        )
        return tuple(outs)

    devices = jax.devices()[:NCORES]
    mesh = bass2jax.Mesh(np.asarray(devices), ("core",))
    P = bass2jax.PartitionSpec("core")
    # PJRT allocates bass_exec custom-call results uninitialized; donating
    # the output operands lets XLA alias them to the results so the NEFF's
    # writes land in the returned buffers (same mechanism run_bass_via_pjrt
    # uses).  Callers chain each dispatch's outputs into the next call's
    # output operands, so no fresh zero buffers are ever uploaded.
    fn = jax.jit(
        bass2jax.shard_map(
            _body, mesh=mesh, in_specs=(P,) * (n_params + len(out_names)),
            out_specs=(P,) * len(out_names), check_rep=False,
        ),
        donate_argnums=tuple(range(n_params, n_params + len(out_names))),
        keep_unused=True,
    )
    return fn, in_names, zero_outs, mesh, P


def _timed_run(pred_inp, enc_inp, W0, b0, W1, b1, reps_inner=33, n_disp=32,
               outer=5):
    """Measure per-execution HW time of the kernel through the axon tunnel.

    The tunnel RTT (~50-100 ms) dwarfs the on-device execution, so a single
    dispatch wall-clock measures the network, not the kernel.  Instead:
    compile two NEFFs -- one with `reps_inner` unrolled kernel bodies, one
    with a single body -- pipeline `n_disp` async dispatches of each, and
    take the marginal time per extra body:

        exec_ns = (T[R reps] - T[1 rep]) / (n_disp * (R - 1))

    Both T's carry identical RTT + per-dispatch overhead, which cancels.
    Every body does the full job: loads inputs from HBM, computes E'/P',
    broadcasts, and writes the entire 64 MB output shard.

    Returns (full_output, exec_ns).
    """
    import time
    import jax
    from concourse import bass2jax

    bass2jax.install_neuronx_cc_hook()

    maps = _in_maps(pred_inp, enc_inp, W0, b0, W1, b1)
    timings = {}
    outs_np = None
    sh = None
    concat_in = cur = None
    for reps in (1, reps_inner):
        key = f"nc{reps}"
        if key not in _cache:
            _cache[key] = _build_nc(reps=reps)
        nc = _cache[key]
        fkey = f"fn{reps}"
        if fkey not in _cache:
            _cache[fkey] = _make_sharded(nc)
        fn, in_names, zero_outs, mesh, P = _cache[fkey]
        if sh is None:
            sh = jax.sharding.NamedSharding(mesh, P)
            concat_in = [
                jax.device_put(
                    np.concatenate([maps[c][nm] for c in range(NCORES)], axis=0),
                    sh,
                )
                for nm in in_names
            ]
            # initial donated output operands; every later dispatch donates
            # the previous dispatch's outputs (the kernel writes every
            # element, so initial contents are irrelevant)
            cur = [
                jax.device_put(
                    np.zeros((NCORES * z.shape[0], *z.shape[1:]), z.dtype), sh
                )
                for z in zero_outs
            ]
            jax.block_until_ready(concat_in)
            jax.block_until_ready(cur)
        # warmup: compile + two execs (first-after-load is unreliable, see
        # kernel() docstring) + correctness snapshot from 1-rep
        cur = list(fn(*concat_in, *cur))
        cur = list(fn(*concat_in, *cur))
        jax.block_until_ready(cur)
        if reps == 1:
            outs_np = np.asarray(cur[0])
        best = None
        for _ in range(outer):
            t0 = time.perf_counter()
            for _i in range(n_disp):
                cur = list(fn(*concat_in, *cur))
            jax.block_until_ready(cur)
            dt = time.perf_counter() - t0
            best = dt if best is None else min(best, dt)
        timings[reps] = best
        if os.environ.get("TIME_DEBUG"):
            print(f"  reps={reps}: best total {best*1e3:.2f} ms "
                  f"({best/n_disp*1e3:.3f} ms/dispatch)")

    exec_ns = (timings[reps_inner] - timings[1]) / (n_disp * (reps_inner - 1)) * 1e9
    return _gather(outs_np), int(exec_ns)
